# revision 4
# baseline (speedup 1.0000x reference)
"""LoRA linear layer (out = x @ (W + s*A@B) + bias) on 8 Trainium2 NeuronCores.

Sharding: data-parallel over rows of x (M = 4*2048 = 8192 -> 1024 rows/core);
each core computes its row-slice against the full weight matrix.

The LoRA update is folded into the weight on the host (W' = W + s*A@B, a
0.5 GFLOP rank-16 update) and everything is cast to bf16 there, so the
device kernel is a pure bf16 GEMM at the PE streaming floor:

  - stationary = W' tile [128k x 128n] bf16, moving = xT [128k x 512m] bf16;
    32 k-tile matmuls accumulate each [128n x 512m] fp32 PSUM tile (out is
    computed transposed; the host transposes it back). bf16 keeps FWL
    weight loads enabled so LDWEIGHTS hides under the 512-cycle matmuls.
  - W' streams in per-n-slab [128 x 32kt x 128n] (1 MiB contiguous DMAs,
    6-deep prefetch) on the qAct HWDGE ring; x and outputs use the qSP
    ring so weight prefetch never queues behind output writeback.
  - x (8 MiB bf16) is SBUF-resident in two rep-parity buffers; each rep
    prefetches the next rep's x mid-pass so back-to-back executions seam
    with no PE gap (keeps the HAM clock-gate warm in steady state).
  - bias is added during the PSUM -> SBUF copy on the scalar engine
    (per-partition bias = per-output-channel in the transposed layout).

bf16 end-to-end max rel err vs the fp32 reference is ~2.4e-3 (8x inside
the 2e-2 gate).
"""
import numpy as np
import ml_dtypes

import concourse.bass as bass
import concourse.tile as tile
from concourse import bacc, mybir
from concourse.bass_utils import run_bass_kernel_spmd

P = 128
N_CORES = 8
BATCH, SEQ = 4, 2048
D_IN, D_OUT, RANK = 4096, 4096, 16
M_FULL = BATCH * SEQ          # 8192
M_C = M_FULL // N_CORES       # 1024 rows per core
KT = D_IN // P                # 32 k-tiles
MC = M_C // 512               # 2 moving chunks of 512
NT = D_OUT // P               # 32 n-tiles (one 128-col W slab each)
XCH = 4                       # k-tiles per x DMA chunk (1 MiB)
NXC = KT // XCH               # 8 x chunks
F32 = mybir.dt.float32
BF16 = mybir.dt.bfloat16
BF16_NP = ml_dtypes.bfloat16

_NC_CACHE = None


def _load_x(nc, xbuf, x_d, start_ch=0, end_ch=NXC):
    """Load x chunks, alternating between the two HWDGE rings."""
    for ch in range(start_ch, end_ch):
        eng = nc.sync if ch % 2 == 0 else nc.scalar
        eng.dma_start(
            out=xbuf[:, ch * XCH:(ch + 1) * XCH, :],
            in_=x_d[:, ch * XCH:(ch + 1) * XCH, :],
        )


def _emit_body(nc, pools, aps, sb, rep, n_reps):
    """Emit one full pass of the kernel (rep > 0 only used for timing)."""
    singles, w_pool, out_pool, psum_pool = pools
    x_d, w_d, bias_d, outt_d = aps
    xT = sb["xT"][rep % 2]
    x_next = sb["xT"][(rep + 1) % 2]

    if rep == 0:
        _load_x(nc, xT, x_d)
        nc.sync.dma_start(out=sb["bias_cols"], in_=bias_d)
    bias_cols = sb["bias_cols"]

    for nt in range(NT):
        wt = w_pool.tile([P, KT, P], BF16, tag="wt", name=f"wt_{rep}_{nt}")
        if rep == 0 and nt == 0:
            # quarter-slab loads so the first matmuls start early
            for q in range(4):
                nc.scalar.dma_start(
                    out=wt[:, q * 8:(q + 1) * 8, :],
                    in_=w_d[0, :, q * 8:(q + 1) * 8, :],
                )
        else:
            nc.scalar.dma_start(out=wt, in_=w_d[nt])
        psums = [psum_pool.tile([P, 512], F32, tag="ps",
                                name=f"ps_{rep}_{nt}_{mc}")
                 for mc in range(MC)]
        for kt in range(KT):
            for mc in range(MC):
                nc.tensor.matmul(
                    psums[mc],
                    wt[:, kt, :],
                    xT[:, kt, mc * 512:(mc + 1) * 512],
                    start=(kt == 0),
                    stop=(kt == KT - 1),
                )
        for mc in range(MC):
            ob = out_pool.tile([P, 512], F32, tag="ob",
                               name=f"ob_{rep}_{nt}_{mc}")
            nc.scalar.activation(
                ob, psums[mc],
                mybir.ActivationFunctionType.Identity,
                bias=bias_cols[:, nt:nt + 1],
            )
            nc.sync.dma_start(
                out=outt_d[nt * P:(nt + 1) * P, mc * 512:(mc + 1) * 512],
                in_=ob,
            )
        # prefetch next rep's x mid-pass (one chunk per nt in 16..23) so
        # the next rep starts with x resident and the PE never idles
        if rep < n_reps - 1 and 16 <= nt < 16 + NXC:
            _load_x(nc, x_next, x_d, start_ch=nt - 16, end_ch=nt - 15)


def _build_nc(n_reps=1):
    nc = bacc.Bacc("TRN2", target_bir_lowering=False, debug=False,
                   num_devices=N_CORES)
    # x slice pre-transposed+tiled on host: [128 p, 32 kt, 1024 m] bf16
    x_d = nc.dram_tensor("xt", [P, KT, M_C], BF16, kind="ExternalInput").ap()
    # W' pre-tiled on host: [32 nt, 128 p, 32 kt, 128 n] bf16 (slab-contig)
    w_d = nc.dram_tensor("w", [NT, P, KT, P], BF16, kind="ExternalInput").ap()
    # bias striped on host: bias_cols[p, nt] = bias[nt*128 + p]
    bias_d = nc.dram_tensor("bias", [P, NT], F32, kind="ExternalInput").ap()
    outt_d = nc.dram_tensor("outt", [D_OUT, M_C], F32,
                            kind="ExternalOutput").ap()

    with tile.TileContext(nc) as tc:
        with (
            tc.tile_pool(name="singles", bufs=1) as singles,
            tc.tile_pool(name="wts", bufs=6) as w_pool,
            tc.tile_pool(name="outs", bufs=6) as out_pool,
            tc.tile_pool(name="psum", bufs=8, space="PSUM") as psum_pool,
        ):
            sb = {
                "xT": [singles.tile([P, KT, M_C], BF16, name="xT0"),
                       singles.tile([P, KT, M_C], BF16, name="xT1")],
                "bias_cols": singles.tile([P, NT], F32, name="bias_cols"),
            }
            pools = (singles, w_pool, out_pool, psum_pool)
            aps = (x_d, w_d, bias_d, outt_d)
            for rep in range(n_reps):
                _emit_body(nc, pools, aps, sb, rep, n_reps)

    nc.compile()
    return nc


def get_nc():
    global _NC_CACHE
    if _NC_CACHE is None:
        _NC_CACHE = _build_nc()
    return _NC_CACHE


def make_in_maps(x, W, bias, lora_A, lora_B, scaling):
    x2 = np.asarray(x, dtype=np.float32).reshape(M_FULL, D_IN)
    s = np.float32(np.asarray(scaling).astype(np.float64))
    a = np.asarray(lora_A, dtype=np.float32)
    b = np.asarray(lora_B, dtype=np.float32)
    wp = (np.asarray(W, dtype=np.float32) + s * (a @ b)).astype(BF16_NP)
    # w_tiled[nt, p, kt, n] = W'[kt*128 + p, nt*128 + n]
    w_tiled = np.ascontiguousarray(
        wp.reshape(KT, P, NT, P).transpose(2, 1, 0, 3))
    bias_cols = np.ascontiguousarray(
        np.asarray(bias, dtype=np.float32).reshape(NT, P).T)
    in_maps = []
    for c in range(N_CORES):
        xt = x2[c * M_C:(c + 1) * M_C].T.astype(BF16_NP)     # [4096, 1024]
        xt_tiled = np.ascontiguousarray(
            xt.reshape(KT, P, M_C).transpose(1, 0, 2))       # [128, 32, 1024]
        in_maps.append({
            "xt": xt_tiled,
            "w": w_tiled,
            "bias": bias_cols,
        })
    return in_maps


def assemble_output(results):
    """results: list of per-core dicts with 'outt' [D_OUT, M_C]."""
    out = np.concatenate(
        [results[c]["outt"].T for c in range(N_CORES)], axis=0)
    return np.ascontiguousarray(out).reshape(BATCH, SEQ, D_OUT)


def kernel(x, W, bias, lora_A, lora_B, scaling):
    nc = get_nc()
    in_maps = make_in_maps(x, W, bias, lora_A, lora_B, scaling)
    res = run_bass_kernel_spmd(nc, in_maps, core_ids=list(range(N_CORES)))
    return assemble_output(res.results)


# revision 5
# speedup vs baseline: 1.4187x; 1.4187x over previous
"""LoRA linear layer (out = x @ (W + s*A@B) + bias) on 8 Trainium2 NeuronCores.

Sharding: data-parallel over rows of x (M = 4*2048 = 8192 -> 1024 rows/core);
each core computes its row-slice against the full weight matrix.

The LoRA update is folded into the weight on the host (W' = W + s*A@B, a
0.5 GFLOP rank-16 update), so the device kernel is a pure GEMM at the PE
streaming floor:

  - k-tiles 0..KTB-1 run in bf16: stationary = W' tile [128k x 128n],
    moving = xT [128k x 512m]; fp32 PSUM accumulation per [128n x 512m]
    tile (out is computed transposed; the host transposes it back).
  - the last KK8 = 512 contraction elements run as fp8-e4m3 DoubleRow
    matmuls (2 fp8 weights/cell, half the cycles): x8 = fp8(x/8) and
    W8 = fp8(8*W') so the products land unscaled and mix directly into
    the same PSUM accumulation as the bf16 part. Max rel err vs the
    fp32 reference ~1.5e-2 (sim 1.46e-2), inside the 2e-2 gate.
  - W'/W8 stream per-n-slab (contiguous DMAs, 6-deep prefetch) on the
    qAct HWDGE ring; x and outputs use the qSP ring so weight prefetch
    never queues behind output writeback.
  - x is SBUF-resident in two rep-parity buffers; each rep prefetches
    the next rep's x mid-pass so back-to-back executions seam with no
    PE gap (keeps the HAM clock-gate warm in steady state).
  - bias is added during the PSUM -> SBUF copy on the scalar engine
    (per-partition bias = per-output-channel in the transposed layout).
"""
import numpy as np
import ml_dtypes

import concourse.bass as bass
import concourse.tile as tile
from concourse import bacc, mybir
from concourse.bass_utils import run_bass_kernel_spmd

P = 128
N_CORES = 8
BATCH, SEQ = 4, 2048
D_IN, D_OUT, RANK = 4096, 4096, 16
M_FULL = BATCH * SEQ          # 8192
M_C = M_FULL // N_CORES       # 1024 rows per core
KK8 = 512                     # contraction tail handled in fp8 DoubleRow
C8 = KK8 // 256               # fp8 double-k chunks (2)
KB = D_IN - KK8               # bf16 contraction prefix (3584)
KTB = KB // P                 # 28 bf16 k-tiles
MC = M_C // 512               # 2 moving chunks of 512
NT = D_OUT // P               # 32 n-tiles (one 128-col W slab each)
XCH = 4                       # k-tiles per x DMA chunk
NXC = KTB // XCH              # 7 x chunks
F32 = mybir.dt.float32
BF16 = mybir.dt.bfloat16
FP8 = mybir.dt.float8e4
BF16_NP = ml_dtypes.bfloat16
FP8_NP = ml_dtypes.float8_e4m3

_NC_CACHE = None


def _load_x(nc, xbuf, x8buf, x_d, x8_d, start_ch=0, end_ch=NXC + 1):
    """Load x chunks (bf16 prefix + fp8 tail), alternating HWDGE rings."""
    for ch in range(start_ch, end_ch):
        eng = nc.sync if ch % 2 == 0 else nc.scalar
        if ch < NXC:
            eng.dma_start(
                out=xbuf[:, ch * XCH:(ch + 1) * XCH, :],
                in_=x_d[:, ch * XCH:(ch + 1) * XCH, :],
            )
        else:
            eng.dma_start(out=x8buf, in_=x8_d)


def _emit_body(nc, pools, aps, sb, rep, n_reps):
    """Emit one full pass of the kernel (rep > 0 only used for timing)."""
    singles, w_pool, w8_pool, out_pool, psum_pool = pools
    x_d, x8_d, w_d, w8_d, bias_d, outt_d = aps
    xT = sb["xT"][rep % 2]
    x8 = sb["x8"][rep % 2]
    x_next = sb["xT"][(rep + 1) % 2]
    x8_next = sb["x8"][(rep + 1) % 2]

    if rep == 0:
        _load_x(nc, xT, x8, x_d, x8_d)
        nc.sync.dma_start(out=sb["bias_cols"], in_=bias_d)
    bias_cols = sb["bias_cols"]

    for nt in range(NT):
        wt = w_pool.tile([P, KTB, P], BF16, tag="wt", name=f"wt_{rep}_{nt}")
        wt8 = w8_pool.tile([P, C8, 2, P], FP8, tag="w8", name=f"w8_{rep}_{nt}")
        if rep == 0 and nt == 0:
            # quarter-slab loads so the first matmuls start early
            for q in range(4):
                nc.scalar.dma_start(
                    out=wt[:, q * 7:(q + 1) * 7, :],
                    in_=w_d[0, :, q * 7:(q + 1) * 7, :],
                )
        else:
            nc.scalar.dma_start(out=wt, in_=w_d[nt])
        nc.scalar.dma_start(out=wt8, in_=w8_d[nt])
        psums = [psum_pool.tile([P, 512], F32, tag="ps",
                                name=f"ps_{rep}_{nt}_{mc}")
                 for mc in range(MC)]
        for kt in range(KTB):
            for mc in range(MC):
                nc.tensor.matmul(
                    psums[mc],
                    wt[:, kt, :],
                    xT[:, kt, mc * 512:(mc + 1) * 512],
                    start=(kt == 0),
                    stop=False,
                )
        for c in range(C8):
            for mc in range(MC):
                nc.tensor.matmul(
                    psums[mc],
                    wt8[:, c, :, :],
                    x8[:, c, :, mc * 512:(mc + 1) * 512],
                    start=False,
                    stop=(c == C8 - 1),
                    perf_mode=mybir.MatmulPerfMode.DoubleRow,
                )
        for mc in range(MC):
            ob = out_pool.tile([P, 512], F32, tag="ob",
                               name=f"ob_{rep}_{nt}_{mc}")
            nc.scalar.activation(
                ob, psums[mc],
                mybir.ActivationFunctionType.Identity,
                bias=bias_cols[:, nt:nt + 1],
            )
            nc.sync.dma_start(
                out=outt_d[nt * P:(nt + 1) * P, mc * 512:(mc + 1) * 512],
                in_=ob,
            )
        # prefetch next rep's x mid-pass (one chunk per nt) so the next
        # rep starts with x resident and the PE never idles
        if rep < n_reps - 1 and 16 <= nt < 16 + NXC + 1:
            _load_x(nc, x_next, x8_next, x_d, x8_d,
                    start_ch=nt - 16, end_ch=nt - 15)


def _build_nc(n_reps=1):
    nc = bacc.Bacc("TRN2", target_bir_lowering=False, debug=False,
                   num_devices=N_CORES)
    # x prefix pre-transposed+tiled on host: [128 p, 28 kt, 1024 m] bf16
    x_d = nc.dram_tensor("xt", [P, KTB, M_C], BF16, kind="ExternalInput").ap()
    # fp8 tail of x: x8[ki, c, ko, m] = fp8(x[m, KB+256c+128ko+ki] / 8)
    x8_d = nc.dram_tensor("x8", [P, C8, 2, M_C], FP8,
                          kind="ExternalInput").ap()
    # W' prefix pre-tiled: [32 nt, 128 p, 28 kt, 128 n] bf16 (slab-contig)
    w_d = nc.dram_tensor("w", [NT, P, KTB, P], BF16, kind="ExternalInput").ap()
    # fp8 tail of W': w8[nt, ki, c, ko, n] = fp8(8 * W'[KB+256c+128ko+ki, .])
    w8_d = nc.dram_tensor("w8", [NT, P, C8, 2, P], FP8,
                          kind="ExternalInput").ap()
    # bias striped on host: bias_cols[p, nt] = bias[nt*128 + p]
    bias_d = nc.dram_tensor("bias", [P, NT], F32, kind="ExternalInput").ap()
    outt_d = nc.dram_tensor("outt", [D_OUT, M_C], F32,
                            kind="ExternalOutput").ap()

    with tile.TileContext(nc) as tc:
        with (
            tc.tile_pool(name="singles", bufs=1) as singles,
            tc.tile_pool(name="wts", bufs=6) as w_pool,
            tc.tile_pool(name="w8s", bufs=6) as w8_pool,
            tc.tile_pool(name="outs", bufs=6) as out_pool,
            tc.tile_pool(name="psum", bufs=8, space="PSUM") as psum_pool,
        ):
            sb = {
                "xT": [singles.tile([P, KTB, M_C], BF16, name="xT0"),
                       singles.tile([P, KTB, M_C], BF16, name="xT1")],
                "x8": [singles.tile([P, C8, 2, M_C], FP8, name="x8_0"),
                       singles.tile([P, C8, 2, M_C], FP8, name="x8_1")],
                "bias_cols": singles.tile([P, NT], F32, name="bias_cols"),
            }
            pools = (singles, w_pool, w8_pool, out_pool, psum_pool)
            aps = (x_d, x8_d, w_d, w8_d, bias_d, outt_d)
            for rep in range(n_reps):
                _emit_body(nc, pools, aps, sb, rep, n_reps)

    nc.compile()
    return nc


def get_nc():
    global _NC_CACHE
    if _NC_CACHE is None:
        _NC_CACHE = _build_nc()
    return _NC_CACHE


def _q8(v):
    return np.clip(v, -240, 240).astype(FP8_NP)


def make_in_maps(x, W, bias, lora_A, lora_B, scaling):
    x2 = np.asarray(x, dtype=np.float32).reshape(M_FULL, D_IN)
    s = np.float32(np.asarray(scaling).astype(np.float64))
    a = np.asarray(lora_A, dtype=np.float32)
    b = np.asarray(lora_B, dtype=np.float32)
    wp = np.asarray(W, dtype=np.float32) + s * (a @ b)
    # w_tiled[nt, p, kt, n] = W'[kt*128 + p, nt*128 + n]   (bf16 prefix)
    w_tiled = np.ascontiguousarray(
        wp[:KB].astype(BF16_NP).reshape(KTB, P, NT, P).transpose(2, 1, 0, 3))
    # w8[nt, ki, c, ko, n] = fp8(8 * W'[KB + 256c + 128ko + ki, nt*128+n])
    w8 = np.ascontiguousarray(
        _q8(8.0 * wp[KB:]).reshape(C8, 2, P, NT, P).transpose(3, 2, 0, 1, 4))
    bias_cols = np.ascontiguousarray(
        np.asarray(bias, dtype=np.float32).reshape(NT, P).T)
    in_maps = []
    for c in range(N_CORES):
        xt = x2[c * M_C:(c + 1) * M_C].T                     # [4096, 1024]
        xt_tiled = np.ascontiguousarray(
            xt[:KB].astype(BF16_NP).reshape(KTB, P, M_C).transpose(1, 0, 2))
        x8_tiled = np.ascontiguousarray(
            _q8(xt[KB:] / 8.0).reshape(C8, 2, P, M_C).transpose(2, 0, 1, 3))
        in_maps.append({
            "xt": xt_tiled,
            "x8": x8_tiled,
            "w": w_tiled,
            "w8": w8,
            "bias": bias_cols,
        })
    return in_maps


def assemble_output(results):
    """results: list of per-core dicts with 'outt' [D_OUT, M_C]."""
    out = np.concatenate(
        [results[c]["outt"].T for c in range(N_CORES)], axis=0)
    return np.ascontiguousarray(out).reshape(BATCH, SEQ, D_OUT)


def kernel(x, W, bias, lora_A, lora_B, scaling):
    nc = get_nc()
    in_maps = make_in_maps(x, W, bias, lora_A, lora_B, scaling)
    res = run_bass_kernel_spmd(nc, in_maps, core_ids=list(range(N_CORES)))
    return assemble_output(res.results)


# revision 6
# speedup vs baseline: 1.5108x; 1.0650x over previous
"""LoRA linear layer (out = x @ (W + s*A@B) + bias) on 8 Trainium2 NeuronCores.

Sharding: data-parallel over rows of x (M = 4*2048 = 8192 -> 1024 rows/core);
each core computes its row-slice against the full weight matrix.

The LoRA update is folded into the weight on the host (W' = W + s*A@B, a
0.5 GFLOP rank-16 update), so the device kernel is a pure GEMM at the PE
streaming floor:

  - k-tiles 0..KTB-1 run in bf16: stationary = W' tile [128k x 128n],
    moving = xT [128k x 512m]; fp32 PSUM accumulation per [128n x 512m]
    tile (out is computed transposed; the host transposes it back).
  - the last KK8 = 512 contraction elements run as fp8-e4m3 DoubleRow
    matmuls (2 fp8 weights/cell, half the cycles): x8 = fp8(x/8) and
    W8 = fp8(8*W') so the products land unscaled and mix directly into
    the same PSUM accumulation as the bf16 part. Max rel err vs the
    fp32 reference ~1.5e-2 (sim 1.46e-2), inside the 2e-2 gate.
  - W'/W8 stream per-n-slab (contiguous DMAs, 6-deep prefetch) on the
    qAct HWDGE ring; x and outputs use the qSP ring so weight prefetch
    never queues behind output writeback.
  - x is SBUF-resident in two rep-parity buffers; each rep prefetches
    the next rep's x mid-pass so back-to-back executions seam with no
    PE gap (keeps the HAM clock-gate warm in steady state).
  - bias is added during the PSUM -> SBUF copy on the scalar engine
    (per-partition bias = per-output-channel in the transposed layout).
"""
import numpy as np
import ml_dtypes

import concourse.tile as tile
from concourse import bacc, mybir
from concourse.bass_utils import run_bass_kernel_spmd

P = 128
N_CORES = 8
BATCH, SEQ = 4, 2048
D_IN, D_OUT, RANK = 4096, 4096, 16
M_FULL = BATCH * SEQ          # 8192
M_C = M_FULL // N_CORES       # 1024 rows per core
KK8 = 512                     # contraction tail handled in fp8 DoubleRow
C8 = KK8 // 256               # fp8 double-k chunks (2)
KB = D_IN - KK8               # bf16 contraction prefix (3584)
KTB = KB // P                 # 28 bf16 k-tiles
MC = M_C // 512               # 2 moving chunks of 512
NT = D_OUT // P               # 32 n-tiles (one 128-col W slab each)
XCH = 4                       # k-tiles per x DMA chunk
NXC = KTB // XCH              # 7 x chunks
F32 = mybir.dt.float32
BF16 = mybir.dt.bfloat16
FP8 = mybir.dt.float8e4
BF16_NP = ml_dtypes.bfloat16
FP8_NP = ml_dtypes.float8_e4m3

_NC_CACHE = None


def _load_x(nc, xbuf, x8buf, x_d, x8_d, start_ch=0, end_ch=NXC + 1):
    """Load x chunks (bf16 prefix + fp8 tail), alternating HWDGE rings."""
    for ch in range(start_ch, end_ch):
        eng = nc.sync if ch % 2 == 0 else nc.scalar
        if ch < NXC:
            eng.dma_start(
                out=xbuf[:, ch * XCH:(ch + 1) * XCH, :],
                in_=x_d[:, ch * XCH:(ch + 1) * XCH, :],
            )
        else:
            eng.dma_start(out=x8buf, in_=x8_d)


def _emit_body(nc, pools, aps, sb, rep, n_reps):
    """Emit one full pass of the kernel (rep > 0 only used for timing)."""
    singles, w_pool, w8_pool, out_pool, psum_pool = pools
    x_d, x8_d, w_d, w8_d, bias_d, outt_d = aps
    xT = sb["xT"][rep % 2]
    x8 = sb["x8"][rep % 2]
    x_next = sb["xT"][(rep + 1) % 2]
    x8_next = sb["x8"][(rep + 1) % 2]

    if rep == 0:
        _load_x(nc, xT, x8, x_d, x8_d)
        nc.sync.dma_start(out=sb["bias_cols"], in_=bias_d)
    bias_cols = sb["bias_cols"]

    for nt in range(NT):
        wt = w_pool.tile([P, KTB, P], BF16, tag="wt", name=f"wt_{rep}_{nt}")
        wt8 = w8_pool.tile([P, C8, 2, P], FP8, tag="w8", name=f"w8_{rep}_{nt}")
        if rep == 0 and nt == 0:
            # quarter-slab loads so the first matmuls start early
            for q in range(4):
                nc.scalar.dma_start(
                    out=wt[:, q * 7:(q + 1) * 7, :],
                    in_=w_d[0, :, q * 7:(q + 1) * 7, :],
                )
        else:
            nc.scalar.dma_start(out=wt, in_=w_d[nt])
        nc.scalar.dma_start(out=wt8, in_=w8_d[nt])
        psums = [psum_pool.tile([P, 512], F32, tag="ps",
                                name=f"ps_{rep}_{nt}_{mc}")
                 for mc in range(MC)]
        for kt in range(KTB):
            for mc in range(MC):
                nc.tensor.matmul(
                    psums[mc],
                    wt[:, kt, :],
                    xT[:, kt, mc * 512:(mc + 1) * 512],
                    start=(kt == 0),
                    stop=False,
                )
        for c in range(C8):
            for mc in range(MC):
                nc.tensor.matmul(
                    psums[mc],
                    wt8[:, c, :, :],
                    x8[:, c, :, mc * 512:(mc + 1) * 512],
                    start=False,
                    stop=(c == C8 - 1),
                    perf_mode=mybir.MatmulPerfMode.DoubleRow,
                )
        for mc in range(MC):
            ob = out_pool.tile([P, 512], F32, tag="ob",
                               name=f"ob_{rep}_{nt}_{mc}")
            nc.scalar.activation(
                ob, psums[mc],
                mybir.ActivationFunctionType.Identity,
                bias=bias_cols[:, nt:nt + 1],
            )
            nc.sync.dma_start(
                out=outt_d[nt * P:(nt + 1) * P, mc * 512:(mc + 1) * 512],
                in_=ob,
            )
        # prefetch next rep's x mid-pass (one chunk per nt) so the next
        # rep starts with x resident and the PE never idles
        if rep < n_reps - 1 and 16 <= nt < 16 + NXC + 1:
            _load_x(nc, x_next, x8_next, x_d, x8_d,
                    start_ch=nt - 16, end_ch=nt - 15)


def _build_nc(n_reps=1):
    nc = bacc.Bacc("TRN2", target_bir_lowering=False, debug=False,
                   num_devices=N_CORES)
    # x prefix pre-transposed+tiled on host: [128 p, 28 kt, 1024 m] bf16
    x_d = nc.dram_tensor("xt", [P, KTB, M_C], BF16, kind="ExternalInput").ap()
    # fp8 tail of x: x8[ki, c, ko, m] = fp8(x[m, KB+256c+128ko+ki] / 8)
    x8_d = nc.dram_tensor("x8", [P, C8, 2, M_C], FP8,
                          kind="ExternalInput").ap()
    # W' prefix pre-tiled: [32 nt, 128 p, 28 kt, 128 n] bf16 (slab-contig)
    w_d = nc.dram_tensor("w", [NT, P, KTB, P], BF16, kind="ExternalInput").ap()
    # fp8 tail of W': w8[nt, ki, c, ko, n] = fp8(8 * W'[KB+256c+128ko+ki, .])
    w8_d = nc.dram_tensor("w8", [NT, P, C8, 2, P], FP8,
                          kind="ExternalInput").ap()
    # bias striped on host: bias_cols[p, nt] = bias[nt*128 + p]
    bias_d = nc.dram_tensor("bias", [P, NT], F32, kind="ExternalInput").ap()
    outt_d = nc.dram_tensor("outt", [D_OUT, M_C], F32,
                            kind="ExternalOutput").ap()

    with tile.TileContext(nc) as tc:
        with (
            tc.tile_pool(name="singles", bufs=1) as singles,
            tc.tile_pool(name="wts", bufs=6) as w_pool,
            tc.tile_pool(name="w8s", bufs=6) as w8_pool,
            tc.tile_pool(name="outs", bufs=6) as out_pool,
            tc.tile_pool(name="psum", bufs=8, space="PSUM") as psum_pool,
        ):
            sb = {
                "xT": [singles.tile([P, KTB, M_C], BF16, name="xT0"),
                       singles.tile([P, KTB, M_C], BF16, name="xT1")],
                "x8": [singles.tile([P, C8, 2, M_C], FP8, name="x8_0"),
                       singles.tile([P, C8, 2, M_C], FP8, name="x8_1")],
                "bias_cols": singles.tile([P, NT], F32, name="bias_cols"),
            }
            pools = (singles, w_pool, w8_pool, out_pool, psum_pool)
            aps = (x_d, x8_d, w_d, w8_d, bias_d, outt_d)
            for rep in range(n_reps):
                _emit_body(nc, pools, aps, sb, rep, n_reps)

    nc.compile()
    return nc


def get_nc():
    global _NC_CACHE
    if _NC_CACHE is None:
        _NC_CACHE = _build_nc()
    return _NC_CACHE


def _q8(v):
    return np.clip(v, -240, 240).astype(FP8_NP)


def make_in_maps(x, W, bias, lora_A, lora_B, scaling):
    x2 = np.asarray(x, dtype=np.float32).reshape(M_FULL, D_IN)
    s = np.float32(np.asarray(scaling).astype(np.float64))
    a = np.asarray(lora_A, dtype=np.float32)
    b = np.asarray(lora_B, dtype=np.float32)
    wp = np.asarray(W, dtype=np.float32) + s * (a @ b)
    # w_tiled[nt, p, kt, n] = W'[kt*128 + p, nt*128 + n]   (bf16 prefix)
    w_tiled = np.ascontiguousarray(
        wp[:KB].astype(BF16_NP).reshape(KTB, P, NT, P).transpose(2, 1, 0, 3))
    # w8[nt, ki, c, ko, n] = fp8(8 * W'[KB + 256c + 128ko + ki, nt*128+n])
    w8 = np.ascontiguousarray(
        _q8(8.0 * wp[KB:]).reshape(C8, 2, P, NT, P).transpose(3, 2, 0, 1, 4))
    bias_cols = np.ascontiguousarray(
        np.asarray(bias, dtype=np.float32).reshape(NT, P).T)
    in_maps = []
    for c in range(N_CORES):
        xt = x2[c * M_C:(c + 1) * M_C].T                     # [4096, 1024]
        xt_tiled = np.ascontiguousarray(
            xt[:KB].astype(BF16_NP).reshape(KTB, P, M_C).transpose(1, 0, 2))
        x8_tiled = np.ascontiguousarray(
            _q8(xt[KB:] / 8.0).reshape(C8, 2, P, M_C).transpose(2, 0, 1, 3))
        in_maps.append({
            "xt": xt_tiled,
            "x8": x8_tiled,
            "w": w_tiled,
            "w8": w8,
            "bias": bias_cols,
        })
    return in_maps


def assemble_output(results):
    """results: list of per-core dicts with 'outt' [D_OUT, M_C]."""
    out = np.concatenate(
        [results[c]["outt"].T for c in range(N_CORES)], axis=0)
    return np.ascontiguousarray(out).reshape(BATCH, SEQ, D_OUT)


def kernel(x, W, bias, lora_A, lora_B, scaling):
    nc = get_nc()
    in_maps = make_in_maps(x, W, bias, lora_A, lora_B, scaling)
    res = run_bass_kernel_spmd(nc, in_maps, core_ids=list(range(N_CORES)))
    return assemble_output(res.results)
